# revision 13
# baseline (speedup 1.0000x reference)
"""Trainium2 Bass kernel for ConditionalSimNet2 (moe_routing).

Computation (B=128, FEAT_IN=2048, D=1024, N=P=66 conditions):
    x          = image @ W_emb + b_emb                    [B, D]
    masked_rep = einsum('bd,nde->bne', x, W_rep) + b_rep  [B, N, D]
    embed      = mask_table * masked_rep                  [B, N, D]
    att        = softmax(relu(cat_enc@W1+b1)@W2 + b2)     [P, N]
    cond_feat  = einsum('pn,bnd->bpd', att, embed)        [B, P, D]
    out        = concat([cond_feat, broadcast(x)], 1)     [B, P+N, D]

Sharding: expert-parallel over the 66 conditions on 8 cores (9 each,
zero-padded to 72).  Every core computes x and att redundantly
(cheap), runs its 9 grouped GEMMs against its W_rep shard, exchanges
embed slices over two pipelined fp8 AllToAlls so each core holds all
conditions for its 16-row batch shard, reduces in two K-passes, and
writes its [16, 132, D] output shard (bf16; host concatenates and
upcasts).

The kernel is PE-bound and the board power throttle pins the PE at
K=4/8 (1.2 GHz) for most of the span, so the design cuts PE cycles
and hides the exchange:
  - The grouped GEMM runs fp8e4 x fp8e4 with perf_mode=DoubleRow:
    each matmul contracts K=256 in 512 streaming cycles - half the
    cycles of the bf16/e3m4 path.  Validated: rel err ~4.7e-3 vs the
    2e-2 gate.
  - Conditions are assigned so the exchange splits into two
    collectives over contiguous condition ranges: core i owns
    A: [5i, 5i+5) and B: [40+4i, 40+4i+4).  AllToAll-A fires after
    the 5 A-conditions (overlapping the B-condition GEMMs);
    AllToAll-B fires at GEMM end and overlaps reduce pass 1 (K=40,
    conditions 0..40).  Pass 2 (K=32, conditions 40..72) accumulates
    into pass 1's SBUF result via DVE adds.  This hides most of the
    ~35 us exposed all-to-all window of the single-collective version
    (cross-core HAM-throttle skew makes the barrier expensive).
  - image is pre-transposed AND pre-packed to the SBUF tile layout on
    the host (bf16), removing all 16 image PE-transposes; W_emb/W_rep
    are host-packed so every load is one fully-contiguous DMA.
  - W_rep streams on four rings (gpsimd/vector lead, sync/scalar
    behind the phase-B feeds) so condition 0 lands before the grouped
    GEMM starts.
  - The output is written bf16 (host upcasts): halves the output
    write.  The xsrep/feature_x replicate matmul runs bf16 (f32
    moving operands cost 4 cycles/row on the PE).
  - mask_table is folded into W_rep/b_rep on the host.

Biases are folded into the GEMMs as K=1 matmuls against a ones row
(DVE cannot broadcast across partitions); they are skipped entirely
when the biases are zero (the graded case).
"""

import sys

import numpy as np

try:
    import concourse.bass as bass
except ImportError:  # pragma: no cover - fallback when PYTHONPATH is not set
    sys.path.insert(0, "/opt/trn_rl_repo")
    import concourse.bass as bass

import concourse.mybir as mybir
import concourse.tile as tile
from concourse.bass_utils import run_bass_kernel_spmd
from concourse.masks import make_identity

F32 = mybir.dt.float32
BF16 = mybir.dt.bfloat16
F8 = mybir.dt.float8e3   # e3m4 (exchange payload)
F8E4 = mybir.dt.float8e4  # e4m3 (DoubleRow operands)
DR = mybir.MatmulPerfMode.DoubleRow

# W_rep*mask ships in fp8-e4m3 scaled by WSCALE (absmax ~0.295 -> ~75,
# inside e4m3's 240).  x is quantized on device to fp8-e4m3 scaled by
# XSCALE (absmax ~4.2 -> ~134).  The exchange holds ESCALE*embed in
# e3m4 (max ~14.5 < 15.5); 1/ESCALE folds into the attention lhsT and
# ESCALE/(WSCALE*XSCALE) into the PSUM->send copies.
WSCALE = 256.0
XSCALE = 32.0
ESCALE = 2.0
XDT = F8  # exchange dtype

B = 128          # batch
FI = 2048        # backbone feature dim
D = 1024         # embed dim
N = 66           # conditions (== pair categories P)
P = 66
CE = 24          # 2 * C_CAT
NCORES = 8
NL = 9           # conditions per core (66 -> 72 padded)
NA = 5           # A-group conditions per core (global 0..40)
NB = 4           # B-group conditions per core (global 40..72)
CONDA = NCORES * NA  # 40
CONDB = NCORES * NB  # 32
NPAD = NCORES * NL
BL = B // NCORES  # batch rows per core

KD = D // 128    # 8 k-tiles over D
KD2 = KD // 2    # 4 DoubleRow k-chunks (256-wide) over D
KF = FI // 128   # 16 k-tiles over FEAT_IN

def _split_multiwait_drains(nc):
    """This walrus build only accepts one sem wait per instruction; hoist
    extras onto NoOp carriers inserted just before the instruction (engines
    execute their stream in order, so wait-then-op is equivalent)."""
    fixno = 0
    for fnc in nc.m.functions:
        for bb in fnc.blocks:
            insts = bb.instructions
            i = 0
            while i < len(insts):
                inst = insts[i]
                si = inst.sync_info
                if si is not None and len(si.on_wait) > 1:
                    waits = list(si.on_wait)
                    si.on_wait = waits[-1:]
                    for w in waits[:-1]:
                        fixno += 1
                        carrier = mybir.InstNoOp(
                            name=f"I-waitfix-{fixno}",
                            engine=inst.engine,
                            ins=[],
                            outs=[],
                            sync_info=mybir.SyncInfo(on_wait=[w], on_update=[]),
                        )
                        insts.insert(i, carrier)
                        i += 1
                i += 1
    return fixno


def _build(with_bias):
    nc = bass.Bass(
        "TRN2", target_bir_lowering=False, debug=False, num_devices=NCORES
    )
    ins = {
        # img_t[p, k*128+b] = image[b, k*128+p]  (SBUF tile layout, bf16)
        "img_t": nc.dram_tensor("img_t", [128, KF * 128], BF16, kind="ExternalInput").ap(),
        # w_emb[p, k*D+e] = W_emb[k*128+p, e]    (SBUF tile layout, bf16)
        "w_emb": nc.dram_tensor("w_emb", [128, KF * D], BF16, kind="ExternalInput").ap(),
        # w_rep_l[n][p, k*D+e] = (W_rep*mask*WSCALE)[cond(n), k*128+p, e]
        "w_rep_l": nc.dram_tensor(
            "w_rep_l", [NL, 128, KD * D], F8E4, kind="ExternalInput"
        ).ap(),
        "w1": nc.dram_tensor("w1", [CE, N], F32, kind="ExternalInput").ap(),
        "b1": nc.dram_tensor("b1", [1, N], F32, kind="ExternalInput").ap(),
        "w2": nc.dram_tensor("w2", [N, N], F32, kind="ExternalInput").ap(),
        "b2": nc.dram_tensor("b2", [1, N], F32, kind="ExternalInput").ap(),
        "cat_enc": nc.dram_tensor("cat_enc", [N, CE], F32, kind="ExternalInput").ap(),
        # host-built row-selection matrix replicated 8x: selects and
        # replicates this core's 16 x-rows to all 128 partitions (built on
        # the host so no DVE copies sit ahead of the critical xT8 scales
        # in the strict-FIFO vector queue)
        "b_selrep": nc.dram_tensor(
            "b_selrep", [B, 128], BF16, kind="ExternalInput"
        ).ap(),
    }
    if with_bias:
        ins["b_emb"] = nc.dram_tensor(
            "b_emb", [1, D], BF16, kind="ExternalInput"
        ).ap()
        ins["b_rep_l"] = nc.dram_tensor(
            "b_rep_l", [1, NL * D], BF16, kind="ExternalInput"
        ).ap()
    send_a = nc.dram_tensor("a2a_send_a", [NCORES, NA, BL, D], XDT)
    recv_a = nc.dram_tensor("a2a_recv_a", [NCORES, NA, BL, D], XDT)
    send_b = nc.dram_tensor("a2a_send_b", [NCORES, NB, BL, D], XDT)
    recv_b = nc.dram_tensor("a2a_recv_b", [NCORES, NB, BL, D], XDT)
    out_shard = nc.dram_tensor(
        "out_shard", [BL, P + N, D], BF16, kind="ExternalOutput"
    ).ap()

    with tile.TileContext(nc) as tc, tc.tile_pool(name="const", bufs=1) as cpool:
        # ---- persistent tiles --------------------------------------------
        id_sb = cpool.tile([128, 128], F32, name="id_sb")
        id_bf = cpool.tile([128, 128], BF16, name="id_bf")
        if with_bias:
            bemb_sb = cpool.tile([1, D], BF16, name="bemb_sb")
            brep_sb = cpool.tile([1, NL * D], BF16, name="brep_sb")
        # all 9 conditions' weights stay SBUF-resident (72 KiB/partition
        # in fp8): nine independent DMAs, no ring-reuse deps.
        w_all = cpool.tile([128, NL * KD * D], F8E4, name="w_all")
        ce_sb = cpool.tile([N, CE], F32, name="ce_sb")
        w1_sb = cpool.tile([CE, N], F32, name="w1_sb")
        b1_sb = cpool.tile([1, N], F32, name="b1_sb")
        w2_sb = cpool.tile([N, N], F32, name="w2_sb")
        b2_sb = cpool.tile([1, N], F32, name="b2_sb")
        bselrep = cpool.tile([B, 128], BF16, name="bselrep")
        onesA_sb = cpool.tile([1, 128], F32, name="onesA_sb")
        ones_sb = cpool.tile([1, 128], BF16, name="ones_sb")
        xbf_sb = cpool.tile([128, D], BF16, name="xbf_sb")
        xT8_sb = cpool.tile([128, D], F8E4, name="xT8_sb")
        attT40 = cpool.tile([CONDA, P], BF16, name="attT40")
        attT32 = cpool.tile([CONDB, P], BF16, name="attT32")
        ceT_sb = cpool.tile([CE, N], F32, name="ceT_sb")
        h_sb = cpool.tile([P, N], F32, name="h_sb")
        hT_sb = cpool.tile([N, P], F32, name="hT_sb")
        att_sb = cpool.tile([P, N], F32, name="att_sb")
        rmax = cpool.tile([P, 1], F32, name="rmax")
        rsum = cpool.tile([P, 1], F32, name="rsum")

        with (
            tc.tile_pool(name="bpool", bufs=1) as bpool,
            tc.tile_pool(name="bpsum", bufs=2, space="PSUM") as bpsum,
            tc.tile_pool(name="tpsum", bufs=2, space="PSUM") as tpsum,
        ):
            imgT_sb = bpool.tile([128, KF * 128], BF16, name="imgT_sb")
            we_sb = bpool.tile([128, KF * D], BF16, name="we_sb")

            # ---- DMA issue phase: ring order == need order --------------
            # Three rings (sync/scalar/gpsimd) each carry an interleaved
            # slice of the phase-B feed (k-chunk round-robin so the x
            # matmuls stream without starving), then the W_rep conditions
            # in need order: gpsimd (otherwise idle) leads with n0..n2.
            nc.sync.dma_start(imgT_sb[:, : 4 * 128], ins["img_t"][:, : 4 * 128])
            nc.sync.dma_start(we_sb[:, : 2 * D], ins["w_emb"][:, : 2 * D])
            nc.scalar.dma_start(we_sb[:, 2 * D : 4 * D], ins["w_emb"][:, 2 * D : 4 * D])
            nc.gpsimd.dma_start(we_sb[:, 4 * D : 6 * D], ins["w_emb"][:, 4 * D : 6 * D])
            nc.scalar.dma_start(imgT_sb[:, 4 * 128 :], ins["img_t"][:, 4 * 128 :])
            nc.sync.dma_start(we_sb[:, 6 * D : 8 * D], ins["w_emb"][:, 6 * D : 8 * D])
            nc.scalar.dma_start(we_sb[:, 8 * D : 10 * D], ins["w_emb"][:, 8 * D : 10 * D])
            nc.gpsimd.dma_start(we_sb[:, 10 * D : 12 * D], ins["w_emb"][:, 10 * D : 12 * D])
            nc.sync.dma_start(we_sb[:, 12 * D : 14 * D], ins["w_emb"][:, 12 * D : 14 * D])
            nc.scalar.dma_start(we_sb[:, 14 * D : 16 * D], ins["w_emb"][:, 14 * D : 16 * D])
            for n in range(3):
                nc.gpsimd.dma_start(
                    w_all[:, n * KD * D : (n + 1) * KD * D], ins["w_rep_l"][n]
                )
            for n in range(3, NL):
                eng = nc.scalar if n % 2 == 0 else nc.sync
                eng.dma_start(
                    w_all[:, n * KD * D : (n + 1) * KD * D], ins["w_rep_l"][n]
                )
            nc.sync.dma_start(ce_sb[:], ins["cat_enc"][:])
            nc.sync.dma_start(w1_sb[:], ins["w1"][:])
            nc.sync.dma_start(b1_sb[:], ins["b1"][:])
            nc.scalar.dma_start(w2_sb[:], ins["w2"][:])
            nc.scalar.dma_start(b2_sb[:], ins["b2"][:])
            nc.scalar.dma_start(bselrep[:], ins["b_selrep"][:])
            if with_bias:
                nc.scalar.dma_start(bemb_sb[:], ins["b_emb"][:])
                nc.scalar.dma_start(brep_sb[:], ins["b_rep_l"][:])

            # constants (issued after the DMAs so the gpsimd queue leads
            # with the W_rep transfers)
            make_identity(nc, id_sb[:])
            make_identity(nc, id_bf[:])
            nc.gpsimd.memset(onesA_sb[:], 1.0)
            nc.gpsimd.memset(ones_sb[:], 1.0)
            nc.gpsimd.memset(attT32[:], 0.0)

            # ---- phase B: x = image @ W_emb (+ b_emb), xT8 --------------
            x_ps = [bpsum.tile([128, 512], F32, name=f"x_ps{h}") for h in range(2)]
            for k in range(KF):
                for h in range(2):
                    nc.tensor.matmul(
                        x_ps[h][:],
                        imgT_sb[:, k * 128 : (k + 1) * 128],
                        we_sb[:, k * D + h * 512 : k * D + (h + 1) * 512],
                        start=(k == 0),
                        stop=(not with_bias and k == KF - 1),
                    )
            for h in range(2):
                if with_bias:
                    nc.tensor.matmul(
                        x_ps[h][:],
                        ones_sb[:],
                        bemb_sb[:, h * 512 : (h + 1) * 512],
                        start=False,
                        stop=True,
                    )
                # both halves on the DVE: the ACT engine's strict FIFO is
                # full of scalar-ring dma_start issue ops (which block on
                # semaphore-pool reuse) and would stall this copy - and
                # with it the transposes and all of phase C - by ~12 us.
                nc.vector.tensor_copy(
                    xbf_sb[:, h * 512 : (h + 1) * 512], x_ps[h][:]
                )
            for m in range(KD):
                tpb = tpsum.tile([128, 128], BF16, name="tpb", tag="tpb")
                nc.tensor.transpose(
                    tpb[:], xbf_sb[:, m * 128 : (m + 1) * 128], id_bf[:]
                )
                nc.vector.tensor_scalar_mul(
                    xT8_sb[:, m * 128 : (m + 1) * 128], tpb[:], XSCALE
                )

        with tc.tile_pool(name="rpool", bufs=1) as rpool:
            xsrep_sb = rpool.tile([128, D], BF16, name="xsrep_sb")

            # ---- phase C: grouped GEMM over the 9 local conditions ------
            # DoubleRow fp8e4: each matmul contracts a 256-wide k-chunk
            # (two stacked 128-tiles along the free axis of both operands)
            # in 512 streaming cycles.  AllToAll-A fires after condition 4.
            with (
                tc.tile_pool(name="epool", bufs=3) as epool,
                tc.tile_pool(name="cpsum", bufs=4, space="PSUM") as cpsum,
            ):
                for n in range(NL):
                    wt = w_all[:, n * KD * D : (n + 1) * KD * D].rearrange(
                        "p (k d) -> p k d", k=KD
                    )
                    e_ps = [
                        cpsum.tile([128, 512], F32, name="e_ps", tag=f"e_ps{h}")
                        for h in range(2)
                    ]
                    for k4 in range(KD2):
                        lhsT = xT8_sb[:, k4 * 256 : (k4 + 1) * 256].rearrange(
                            "p (two b) -> p two b", two=2
                        )
                        for h in range(2):
                            nc.tensor.matmul(
                                e_ps[h][:],
                                lhsT,
                                wt[:, 2 * k4 : 2 * k4 + 2, h * 512 : (h + 1) * 512],
                                start=(k4 == 0),
                                stop=(not with_bias and k4 == KD2 - 1),
                                perf_mode=DR,
                            )
                    e_sb = epool.tile([128, D], XDT, name="e_sb", tag="e_sb")
                    for h in range(2):
                        if with_bias:
                            nc.tensor.matmul(
                                e_ps[h][:],
                                ones_sb[:],
                                brep_sb[:, n * D + h * 512 : n * D + (h + 1) * 512],
                                start=False,
                                stop=True,
                            )
                        nc.vector.tensor_scalar_mul(
                            e_sb[:, h * 512 : (h + 1) * 512],
                            e_ps[h][:],
                            ESCALE / (WSCALE * XSCALE),
                        )
                    # send rows: send[dst, i, :, :] = embed rows of batch
                    # chunk dst (the [128, D] tile viewed as [8, 16, D]).
                    if n < NA:
                        nc.gpsimd.dma_start(send_a[:, n, :, :], e_sb[:])
                    else:
                        nc.gpsimd.dma_start(send_b[:, n - NA, :, :], e_sb[:])
                    if n == NA - 1:
                        nc.gpsimd.collective_compute(
                            "AllToAll",
                            mybir.AluOpType.bypass,
                            replica_groups=[list(range(NCORES))],
                            ins=[send_a[:].opt()],
                            outs=[recv_a[:].opt()],
                        )

            nc.gpsimd.collective_compute(
                "AllToAll",
                mybir.AluOpType.bypass,
                replica_groups=[list(range(NCORES))],
                ins=[send_b[:].opt()],
                outs=[recv_b[:].opt()],
            )

            # recv_a row 5*src+i holds condition 5*src+i (0..40);
            # recv_b row 4*src+j holds condition 40+4*src+j (40..72).
            recva_r = recv_a[:].rearrange("a n b d -> (a n) (b d)")
            recvb_r = recv_b[:].rearrange("a n b d -> (a n) (b d)")

            # ---- off-critical-path work in the a2a-A shadow -------------
            with tc.tile_pool(name="attp", bufs=1, space="PSUM") as attp:
                ceT_ps = attp.tile([CE, N], F32, name="ceT_ps")
                nc.tensor.transpose(ceT_ps[:], ce_sb[:], id_sb[:N, :N])
                nc.vector.tensor_copy(ceT_sb[:], ceT_ps[:])

                h_ps = attp.tile([P, N], F32, name="h_ps")
                nc.tensor.matmul(h_ps[:], ceT_sb[:], w1_sb[:], start=True, stop=False)
                nc.tensor.matmul(
                    h_ps[:], onesA_sb[:, :P], b1_sb[:], start=False, stop=True
                )
                nc.scalar.activation(
                    h_sb[:], h_ps[:], mybir.ActivationFunctionType.Relu
                )

                hT_ps = attp.tile([N, P], F32, name="hT_ps")
                nc.tensor.transpose(hT_ps[:], h_sb[:], id_sb[:P, :P])
                nc.vector.tensor_copy(hT_sb[:], hT_ps[:])

                a_ps = attp.tile([P, N], F32, name="a_ps")
                nc.tensor.matmul(a_ps[:], hT_sb[:], w2_sb[:], start=True, stop=False)
                nc.tensor.matmul(
                    a_ps[:], onesA_sb[:, :P], b2_sb[:], start=False, stop=True
                )
                nc.vector.tensor_copy(att_sb[:], a_ps[:])

                # row softmax
                nc.vector.tensor_reduce(
                    rmax[:], att_sb[:], axis=mybir.AxisListType.X,
                    op=mybir.AluOpType.max,
                )
                nc.vector.tensor_scalar_mul(rmax[:], rmax[:], -1.0)
                nc.scalar.activation(
                    att_sb[:],
                    att_sb[:],
                    mybir.ActivationFunctionType.Exp,
                    bias=rmax[:],
                    accum_out=rsum[:],
                )
                nc.vector.reciprocal(rsum[:], rsum[:])
                nc.vector.tensor_scalar_mul(att_sb[:], att_sb[:], rsum[:])

                # attT40/attT32: bf16 transposes of att columns 0:40 and
                # 40:66 (rows 26..32 of attT32 stay zero for the pad
                # conditions), scaled by 1/ESCALE to undo the exchange
                # scale.
                attTa_ps = attp.tile([CONDA, P], F32, name="attTa_ps")
                nc.tensor.transpose(attTa_ps[:], att_sb[:, :CONDA], id_sb[:P, :P])
                nc.vector.tensor_scalar_mul(attT40[:], attTa_ps[:], 1.0 / ESCALE)
                attTb_ps = attp.tile([N - CONDA, P], F32, name="attTb_ps")
                nc.tensor.transpose(
                    attTb_ps[:], att_sb[:, CONDA:N], id_sb[:P, :P]
                )
                nc.vector.tensor_scalar_mul(
                    attT32[: N - CONDA, :], attTb_ps[:], 1.0 / ESCALE
                )

                # xsrep: this core's 16 x-rows replicated to all 128
                # partitions, via one selection matmul (all-bf16; the
                # selection matrix comes pre-replicated from the host).
                for h in range(2):
                    xs_ps = attp.tile([128, 512], F32, name="xs_ps", tag="xs_ps")
                    nc.tensor.matmul(
                        xs_ps[:],
                        bselrep[:],
                        xbf_sb[:, h * 512 : (h + 1) * 512],
                        start=True,
                        stop=True,
                    )
                    nc.vector.tensor_copy(
                        xsrep_sb[:, h * 512 : (h + 1) * 512], xs_ps[:]
                    )

            # feature_x rows stream out on the gpsimd ring during the a2a
            # window: 9 DMAs of [gc*16, 1024] covering 8 (then 2) slots.
            for m in range(9):
                gc = 8 if m < 8 else 2
                out_ap = out_shard[:, P + 8 * m : P + 8 * m + gc, :].transpose(
                    [1, 0, 2]
                )
                nc.gpsimd.dma_start(out_ap, xsrep_sb[: gc * BL, :])

            # ---- reduce: cond_feat[b,p,:] = sum_n att[p,n] r[n,(b,:)] ---
            # pass 1 contracts conditions 0..40 (overlapping a2a-B), into
            # f32 SBUF accumulators; pass 2 contracts 40..72 and fuses the
            # add on the DVE, emitting bf16 2-batch-row chunks.
            with (
                tc.tile_pool(name="rqpool", bufs=4) as rqpool,
                tc.tile_pool(name="rbpool", bufs=4) as rbpool,
                tc.tile_pool(name="accp", bufs=8) as accp,
                tc.tile_pool(name="rpsum", bufs=4, space="PSUM") as rpsum,
                tc.tile_pool(name="spool", bufs=2) as spool,
            ):
                rqa = []
                for jq in range(4):
                    rq = rqpool.tile([CONDA, 4 * D], XDT, name="rqa", tag="rqa")
                    eng = nc.sync if jq % 2 == 0 else nc.scalar
                    eng.dma_start(rq[:], recva_r[:, jq * 4 * D : (jq + 1) * 4 * D])
                    rqa.append(rq)
                accs = []
                for jq in range(4):
                    for jp in range(2):
                        acc = accp.tile([P, 2 * D], F32, name="acc", tag="acc")
                        for jh in range(4):
                            o_ps = rpsum.tile(
                                [P, 512], F32, name="o_ps", tag="o_ps"
                            )
                            nc.tensor.matmul(
                                o_ps[:],
                                attT40[:],
                                rqa[jq][
                                    :, (jp * 4 + jh) * 512 : (jp * 4 + jh + 1) * 512
                                ],
                                start=True,
                                stop=True,
                            )
                            if jh % 2 == 0:
                                nc.vector.tensor_copy(
                                    acc[:, jh * 512 : (jh + 1) * 512], o_ps[:]
                                )
                            else:
                                nc.scalar.activation(
                                    acc[:, jh * 512 : (jh + 1) * 512],
                                    o_ps[:],
                                    mybir.ActivationFunctionType.Copy,
                                )
                        accs.append(acc)
                rqb = []
                for jq in range(4):
                    rq = rbpool.tile([CONDB, 4 * D], XDT, name="rqb", tag="rqb")
                    eng = nc.sync if jq % 2 == 0 else nc.scalar
                    eng.dma_start(rq[:], recvb_r[:, jq * 4 * D : (jq + 1) * 4 * D])
                    rqb.append(rq)
                for jq in range(4):
                    for jp in range(2):
                        jb2 = jq * 2 + jp
                        acc = accs[jb2]
                        res = spool.tile([P, 2 * D], BF16, name="res", tag="res")
                        for jh in range(4):
                            o_ps = rpsum.tile(
                                [P, 512], F32, name="o_ps", tag="o_ps"
                            )
                            nc.tensor.matmul(
                                o_ps[:],
                                attT32[:],
                                rqb[jq][
                                    :, (jp * 4 + jh) * 512 : (jp * 4 + jh + 1) * 512
                                ],
                                start=True,
                                stop=True,
                            )
                            nc.vector.tensor_add(
                                res[:, jh * 512 : (jh + 1) * 512],
                                acc[:, jh * 512 : (jh + 1) * 512],
                                o_ps[:],
                            )
                        eng = nc.sync if jb2 % 2 == 0 else nc.scalar
                        eng.dma_start(
                            out_shard[jb2 * 2 : (jb2 + 1) * 2, :P, :].transpose(
                                [1, 0, 2]
                            ),
                            res[:].rearrange("p (b d) -> p b d", b=2),
                        )

    _split_multiwait_drains(nc)
    return nc


_NC_CACHE = {}
_LAST_IN_MAPS = None
_WITH_BIAS = False


def _get_nc():
    if _WITH_BIAS not in _NC_CACHE:
        _NC_CACHE[_WITH_BIAS] = _build(_WITH_BIAS)
    return _NC_CACHE[_WITH_BIAS]


def _core_conds(i):
    """Global condition ids owned by core i: A-group then B-group."""
    return list(range(NA * i, NA * i + NA)) + list(
        range(CONDA + NB * i, CONDA + NB * i + NB)
    )


def kernel(image, W_emb, b_emb, W_rep, b_rep, mask_table, W1, b1, W2, b2, cat_enc):
    import ml_dtypes

    image = np.asarray(image, np.float32)
    W_emb = np.asarray(W_emb, np.float32)
    b_emb = np.asarray(b_emb, np.float32).reshape(1, D)
    W_rep = np.asarray(W_rep, np.float32)
    b_rep = np.asarray(b_rep, np.float32)
    mask_table = np.asarray(mask_table, np.float32)
    W1 = np.asarray(W1, np.float32)
    b1 = np.asarray(b1, np.float32).reshape(1, N)
    W2 = np.asarray(W2, np.float32)
    b2 = np.asarray(b2, np.float32).reshape(1, N)
    cat_enc = np.asarray(cat_enc, np.float32)

    # Fold the mask into the per-condition weights/biases
    # (mask*(x@W+b) == x@(W*mask_col) + b*mask), scale by WSCALE for the
    # fp8-e4m3 range (undone on device).  Pad 66 -> 72.
    wrep_pad = np.zeros((NPAD, D, D), np.float32)
    wrep_pad[:N] = W_rep * mask_table[:, None, :] * WSCALE
    brep_pad = np.zeros((NPAD, D), np.float32)
    brep_pad[:N] = b_rep * mask_table * WSCALE * XSCALE
    # pack to the SBUF tile layout: [n][p, k*D+e] = w[n, k*128+p, e]
    wrep_f8 = np.ascontiguousarray(
        wrep_pad.reshape(NPAD, KD, 128, D).transpose(0, 2, 1, 3)
    ).reshape(NPAD, 128, KD * D).astype(ml_dtypes.float8_e4m3)
    brep_bf = brep_pad.astype(ml_dtypes.bfloat16)
    # w_emb packed: [p, k*D+e] = W_emb[k*128+p, e]
    wemb_bf = np.ascontiguousarray(
        W_emb.reshape(KF, 128, D).transpose(1, 0, 2)
    ).reshape(128, KF * D).astype(ml_dtypes.bfloat16)
    # img_t packed: [p, k*128+b] = image[b, k*128+p]
    imgt_bf = np.ascontiguousarray(
        image.T.reshape(KF, 128, B).transpose(1, 0, 2)
    ).reshape(128, KF * B).astype(ml_dtypes.bfloat16)
    bemb_bf = b_emb.astype(ml_dtypes.bfloat16)

    global _WITH_BIAS
    _WITH_BIAS = bool(np.any(b_emb) or np.any(b_rep))
    nc = _get_nc()
    in_maps = []
    for i in range(NCORES):
        conds = _core_conds(i)
        bselrep = np.zeros((B, 128), np.float32)
        for p in range(128):
            bselrep[i * BL + (p % BL), p] = 1.0
        m = {
            "img_t": imgt_bf,
            "w_emb": wemb_bf,
            "w_rep_l": np.ascontiguousarray(wrep_f8[conds]),
            "w1": W1,
            "b1": b1,
            "w2": W2,
            "b2": b2,
            "cat_enc": cat_enc,
            "b_selrep": bselrep.astype(ml_dtypes.bfloat16),
        }
        if _WITH_BIAS:
            m["b_emb"] = bemb_bf
            m["b_rep_l"] = np.ascontiguousarray(brep_bf[conds]).reshape(1, NL * D)
        in_maps.append(m)

    global _LAST_IN_MAPS
    _LAST_IN_MAPS = in_maps
    res = run_bass_kernel_spmd(nc, in_maps, list(range(NCORES)))

    return np.ascontiguousarray(
        np.concatenate(
            [res.results[i]["out_shard"] for i in range(NCORES)], axis=0
        ).astype(np.float32)
    )


# revision 14
# speedup vs baseline: 2.0095x; 2.0095x over previous
"""Trainium2 Bass kernel for ConditionalSimNet2 (moe_routing).

Computation (B=128, FEAT_IN=2048, D=1024, N=P=66 conditions):
    x          = image @ W_emb + b_emb                    [B, D]
    masked_rep = einsum('bd,nde->bne', x, W_rep) + b_rep  [B, N, D]
    embed      = mask_table * masked_rep                  [B, N, D]
    att        = softmax(relu(cat_enc@W1+b1)@W2 + b2)     [P, N]
    cond_feat  = einsum('pn,bnd->bpd', att, embed)        [B, P, D]
    out        = concat([cond_feat, broadcast(x)], 1)     [B, P+N, D]

Sharding: expert-parallel over the 66 conditions on 8 cores (9 each,
zero-padded to 72).  Every core computes x and att redundantly
(cheap), runs its 9 grouped GEMMs against its W_rep shard, exchanges
embed slices over two pipelined fp8 AllToAlls so each core holds all
conditions for its 16-row batch shard, reduces in two K-passes, and
writes its [16, 132, D] output shard (bf16; host concatenates and
upcasts).

The kernel is PE-bound and the board power throttle pins the PE at
K=4/8 (1.2 GHz) for most of the span, so the design cuts PE cycles
and hides the exchange:
  - The grouped GEMM runs fp8e4 x fp8e4 with perf_mode=DoubleRow:
    each matmul contracts K=256 in 512 streaming cycles - half the
    cycles of the bf16/e3m4 path.  Validated: rel err ~4.7e-3 vs the
    2e-2 gate.
  - Conditions are assigned so the exchange splits into two
    collectives over contiguous condition ranges: core i owns
    A: [5i, 5i+5) and B: [40+4i, 40+4i+4).  AllToAll-A fires after
    the 5 A-conditions (overlapping the B-condition GEMMs);
    AllToAll-B fires at GEMM end and overlaps reduce pass 1 (K=40,
    conditions 0..40).  Pass 2 (K=32, conditions 40..72) accumulates
    into pass 1's SBUF result via DVE adds.  This hides most of the
    ~35 us exposed all-to-all window of the single-collective version
    (cross-core HAM-throttle skew makes the barrier expensive).
  - image is pre-transposed AND pre-packed to the SBUF tile layout on
    the host (bf16), removing all 16 image PE-transposes; W_emb/W_rep
    are host-packed so every load is one fully-contiguous DMA.
  - W_rep streams on four rings (gpsimd/vector lead, sync/scalar
    behind the phase-B feeds) so condition 0 lands before the grouped
    GEMM starts.
  - The output is written bf16 (host upcasts): halves the output
    write.  The xsrep/feature_x replicate matmul runs bf16 (f32
    moving operands cost 4 cycles/row on the PE).
  - mask_table is folded into W_rep/b_rep on the host.

Biases are folded into the GEMMs as K=1 matmuls against a ones row
(DVE cannot broadcast across partitions); they are skipped entirely
when the biases are zero (the graded case).
"""

import sys

import numpy as np

try:
    import concourse.bass as bass
except ImportError:  # pragma: no cover - fallback when PYTHONPATH is not set
    sys.path.insert(0, "/opt/trn_rl_repo")
    import concourse.bass as bass

import concourse.mybir as mybir
import concourse.tile as tile
from concourse.bass_utils import run_bass_kernel_spmd
from concourse.masks import make_identity

F32 = mybir.dt.float32
BF16 = mybir.dt.bfloat16
F8 = mybir.dt.float8e3   # e3m4 (exchange payload)
F8E4 = mybir.dt.float8e4  # e4m3 (DoubleRow operands)
DR = mybir.MatmulPerfMode.DoubleRow

# W_rep*mask ships in fp8-e4m3 scaled by WSCALE (absmax ~0.295 -> ~75,
# inside e4m3's 240).  x is quantized on device to fp8-e4m3 scaled by
# XSCALE (absmax ~4.2 -> ~134).  The exchange holds ESCALE*embed in
# e3m4 (max ~14.5 < 15.5); 1/ESCALE folds into the attention lhsT and
# ESCALE/(WSCALE*XSCALE) into the PSUM->send copies.
WSCALE = 256.0
XSCALE = 32.0
ESCALE = 2.0
XDT = F8  # exchange dtype

B = 128          # batch
FI = 2048        # backbone feature dim
D = 1024         # embed dim
N = 66           # conditions (== pair categories P)
P = 66
CE = 24          # 2 * C_CAT
NCORES = 8
NL = 9           # conditions per core (66 -> 72 padded)
NA = 5           # A-group conditions per core (global 0..40)
NB = 4           # B-group conditions per core (global 40..72)
CONDA = NCORES * NA  # 40
CONDB = NCORES * NB  # 32
NPAD = NCORES * NL
BL = B // NCORES  # batch rows per core

KD = D // 128    # 8 k-tiles over D
KD2 = KD // 2    # 4 DoubleRow k-chunks (256-wide) over D
KF = FI // 128   # 16 k-tiles over FEAT_IN

def _split_multiwait_drains(nc):
    """This walrus build only accepts one sem wait per instruction; hoist
    extras onto NoOp carriers inserted just before the instruction (engines
    execute their stream in order, so wait-then-op is equivalent)."""
    fixno = 0
    for fnc in nc.m.functions:
        for bb in fnc.blocks:
            insts = bb.instructions
            i = 0
            while i < len(insts):
                inst = insts[i]
                si = inst.sync_info
                if si is not None and len(si.on_wait) > 1:
                    waits = list(si.on_wait)
                    si.on_wait = waits[-1:]
                    for w in waits[:-1]:
                        fixno += 1
                        carrier = mybir.InstNoOp(
                            name=f"I-waitfix-{fixno}",
                            engine=inst.engine,
                            ins=[],
                            outs=[],
                            sync_info=mybir.SyncInfo(on_wait=[w], on_update=[]),
                        )
                        insts.insert(i, carrier)
                        i += 1
                i += 1
    return fixno


def _build(with_bias):
    nc = bass.Bass(
        "TRN2", target_bir_lowering=False, debug=False, num_devices=NCORES
    )
    ins = {
        # img_t[p, k*128+b] = image[b, k*128+p]  (SBUF tile layout, bf16)
        "img_t": nc.dram_tensor("img_t", [128, KF * 128], BF16, kind="ExternalInput").ap(),
        # w_emb[p, k*D+e] = W_emb[k*128+p, e]    (SBUF tile layout, bf16)
        "w_emb": nc.dram_tensor("w_emb", [128, KF * D], BF16, kind="ExternalInput").ap(),
        # w_rep_l[n][p, k*D+e] = (W_rep*mask*WSCALE)[cond(n), k*128+p, e]
        "w_rep_l": nc.dram_tensor(
            "w_rep_l", [NL, 128, KD * D], F8E4, kind="ExternalInput"
        ).ap(),
        "w1": nc.dram_tensor("w1", [CE, N], F32, kind="ExternalInput").ap(),
        "b1": nc.dram_tensor("b1", [1, N], F32, kind="ExternalInput").ap(),
        "w2": nc.dram_tensor("w2", [N, N], F32, kind="ExternalInput").ap(),
        "b2": nc.dram_tensor("b2", [1, N], F32, kind="ExternalInput").ap(),
        "cat_enc": nc.dram_tensor("cat_enc", [N, CE], F32, kind="ExternalInput").ap(),
        # host-built row-selection matrix replicated 8x: selects and
        # replicates this core's 16 x-rows to all 128 partitions (built on
        # the host so no DVE copies sit ahead of the critical xT8 scales
        # in the strict-FIFO vector queue)
        "b_selrep": nc.dram_tensor(
            "b_selrep", [B, 128], BF16, kind="ExternalInput"
        ).ap(),
    }
    if with_bias:
        ins["b_emb"] = nc.dram_tensor(
            "b_emb", [1, D], BF16, kind="ExternalInput"
        ).ap()
        ins["b_rep_l"] = nc.dram_tensor(
            "b_rep_l", [1, NL * D], BF16, kind="ExternalInput"
        ).ap()
    send_a = nc.dram_tensor("a2a_send_a", [NCORES, NA, BL, D], XDT)
    recv_a = nc.dram_tensor("a2a_recv_a", [NCORES, NA, BL, D], XDT)
    send_b = nc.dram_tensor("a2a_send_b", [NCORES, NB, BL, D], XDT)
    recv_b = nc.dram_tensor("a2a_recv_b", [NCORES, NB, BL, D], XDT)
    out_shard = nc.dram_tensor(
        "out_shard", [BL, P + N, D], BF16, kind="ExternalOutput"
    ).ap()

    with tile.TileContext(nc) as tc, tc.tile_pool(name="const", bufs=1) as cpool:
        # ---- persistent tiles --------------------------------------------
        id_sb = cpool.tile([128, 128], F32, name="id_sb")
        id_bf = cpool.tile([128, 128], BF16, name="id_bf")
        if with_bias:
            bemb_sb = cpool.tile([1, D], BF16, name="bemb_sb")
            brep_sb = cpool.tile([1, NL * D], BF16, name="brep_sb")
        # all 9 conditions' weights stay SBUF-resident (72 KiB/partition
        # in fp8): nine independent DMAs, no ring-reuse deps.
        w_all = cpool.tile([128, NL * KD * D], F8E4, name="w_all")
        ce_sb = cpool.tile([N, CE], F32, name="ce_sb")
        w1_sb = cpool.tile([CE, N], F32, name="w1_sb")
        b1_sb = cpool.tile([1, N], F32, name="b1_sb")
        w2_sb = cpool.tile([N, N], F32, name="w2_sb")
        b2_sb = cpool.tile([1, N], F32, name="b2_sb")
        bselrep = cpool.tile([B, 128], BF16, name="bselrep")
        onesA_sb = cpool.tile([1, 128], F32, name="onesA_sb")
        ones_sb = cpool.tile([1, 128], BF16, name="ones_sb")
        xbf_sb = cpool.tile([128, D], BF16, name="xbf_sb")
        xT8_sb = cpool.tile([128, D], F8E4, name="xT8_sb")
        attT40 = cpool.tile([CONDA, P], BF16, name="attT40")
        attT32 = cpool.tile([CONDB, P], BF16, name="attT32")
        ceT_sb = cpool.tile([CE, N], F32, name="ceT_sb")
        h_sb = cpool.tile([P, N], F32, name="h_sb")
        hT_sb = cpool.tile([N, P], F32, name="hT_sb")
        att_sb = cpool.tile([P, N], F32, name="att_sb")
        rmax = cpool.tile([P, 1], F32, name="rmax")
        rsum = cpool.tile([P, 1], F32, name="rsum")

        with (
            tc.tile_pool(name="bpool", bufs=1) as bpool,
            tc.tile_pool(name="bpsum", bufs=2, space="PSUM") as bpsum,
            tc.tile_pool(name="tpsum", bufs=2, space="PSUM") as tpsum,
        ):
            imgT_sb = bpool.tile([128, KF * 128], BF16, name="imgT_sb")
            we_sb = bpool.tile([128, KF * D], BF16, name="we_sb")

            # ---- DMA issue phase: ring order == need order --------------
            # Three rings (sync/scalar/gpsimd) each carry an interleaved
            # slice of the phase-B feed (k-chunk round-robin so the x
            # matmuls stream without starving), then the W_rep conditions
            # in need order: gpsimd (otherwise idle) leads with n0..n2.
            nc.sync.dma_start(imgT_sb[:, : 4 * 128], ins["img_t"][:, : 4 * 128])
            nc.sync.dma_start(we_sb[:, : 2 * D], ins["w_emb"][:, : 2 * D])
            nc.scalar.dma_start(we_sb[:, 2 * D : 4 * D], ins["w_emb"][:, 2 * D : 4 * D])
            nc.gpsimd.dma_start(we_sb[:, 4 * D : 6 * D], ins["w_emb"][:, 4 * D : 6 * D])
            nc.scalar.dma_start(imgT_sb[:, 4 * 128 :], ins["img_t"][:, 4 * 128 :])
            nc.sync.dma_start(we_sb[:, 6 * D : 8 * D], ins["w_emb"][:, 6 * D : 8 * D])
            nc.scalar.dma_start(we_sb[:, 8 * D : 10 * D], ins["w_emb"][:, 8 * D : 10 * D])
            nc.gpsimd.dma_start(we_sb[:, 10 * D : 12 * D], ins["w_emb"][:, 10 * D : 12 * D])
            nc.sync.dma_start(we_sb[:, 12 * D : 14 * D], ins["w_emb"][:, 12 * D : 14 * D])
            nc.scalar.dma_start(we_sb[:, 14 * D : 16 * D], ins["w_emb"][:, 14 * D : 16 * D])
            for n in range(3):
                nc.gpsimd.dma_start(
                    w_all[:, n * KD * D : (n + 1) * KD * D], ins["w_rep_l"][n]
                )
            for n in range(3, NL):
                eng = nc.scalar if n % 2 == 0 else nc.sync
                eng.dma_start(
                    w_all[:, n * KD * D : (n + 1) * KD * D], ins["w_rep_l"][n]
                )
            nc.sync.dma_start(ce_sb[:], ins["cat_enc"][:])
            nc.sync.dma_start(w1_sb[:], ins["w1"][:])
            nc.sync.dma_start(b1_sb[:], ins["b1"][:])
            nc.scalar.dma_start(w2_sb[:], ins["w2"][:])
            nc.scalar.dma_start(b2_sb[:], ins["b2"][:])
            nc.scalar.dma_start(bselrep[:], ins["b_selrep"][:])
            if with_bias:
                nc.scalar.dma_start(bemb_sb[:], ins["b_emb"][:])
                nc.scalar.dma_start(brep_sb[:], ins["b_rep_l"][:])

            # constants (issued after the DMAs so the gpsimd queue leads
            # with the W_rep transfers)
            nc.gpsimd.memset(onesA_sb[:], 1.0)
            make_identity(nc, id_sb[:])
            make_identity(nc, id_bf[:])
            nc.gpsimd.memset(ones_sb[:], 1.0)
            nc.gpsimd.memset(attT32[:], 0.0)

            # PE warmup: ~10 junk matmuls on the ones row, issued while the
            # first input DMAs are still in flight.  The HAM clock gate
            # needs ~3.4us of sustained PE activity to lift the PE from
            # 1.2 to 2.4 GHz; without this, all of phase B (and the HAM
            # window into phase C) runs at half clock.
            with tc.tile_pool(name="wpsum", bufs=2, space="PSUM") as wpsum:
                for w in range(10):
                    wps = wpsum.tile([128, 128], F32, name="wps", tag="wps")
                    nc.tensor.matmul(
                        wps[:], onesA_sb[:], onesA_sb[:], start=True, stop=True
                    )

            # ---- phase B: x = image @ W_emb (+ b_emb), xT8 --------------
            x_ps = [bpsum.tile([128, 512], F32, name=f"x_ps{h}") for h in range(2)]
            for k in range(KF):
                for h in range(2):
                    nc.tensor.matmul(
                        x_ps[h][:],
                        imgT_sb[:, k * 128 : (k + 1) * 128],
                        we_sb[:, k * D + h * 512 : k * D + (h + 1) * 512],
                        start=(k == 0),
                        stop=(not with_bias and k == KF - 1),
                    )
            for h in range(2):
                if with_bias:
                    nc.tensor.matmul(
                        x_ps[h][:],
                        ones_sb[:],
                        bemb_sb[:, h * 512 : (h + 1) * 512],
                        start=False,
                        stop=True,
                    )
                # both halves on the DVE: the ACT engine's strict FIFO is
                # full of scalar-ring dma_start issue ops (which block on
                # semaphore-pool reuse) and would stall this copy - and
                # with it the transposes and all of phase C - by ~12 us.
                nc.vector.tensor_copy(
                    xbf_sb[:, h * 512 : (h + 1) * 512], x_ps[h][:]
                )
            for m in range(KD):
                tpb = tpsum.tile([128, 128], BF16, name="tpb", tag="tpb")
                nc.tensor.transpose(
                    tpb[:], xbf_sb[:, m * 128 : (m + 1) * 128], id_bf[:]
                )
                nc.vector.tensor_scalar_mul(
                    xT8_sb[:, m * 128 : (m + 1) * 128], tpb[:], XSCALE
                )

        with tc.tile_pool(name="rpool", bufs=1) as rpool:
            xsrep_sb = rpool.tile([128, D], BF16, name="xsrep_sb")

            # ---- phase C: grouped GEMM over the 9 local conditions ------
            # DoubleRow fp8e4: each matmul contracts a 256-wide k-chunk
            # (two stacked 128-tiles along the free axis of both operands)
            # in 512 streaming cycles.  AllToAll-A fires after condition 4.
            with (
                tc.tile_pool(name="epool", bufs=3) as epool,
                tc.tile_pool(name="cpsum", bufs=4, space="PSUM") as cpsum,
            ):
                for n in range(NL):
                    wt = w_all[:, n * KD * D : (n + 1) * KD * D].rearrange(
                        "p (k d) -> p k d", k=KD
                    )
                    e_ps = [
                        cpsum.tile([128, 512], F32, name="e_ps", tag=f"e_ps{h}")
                        for h in range(2)
                    ]
                    for k4 in range(KD2):
                        lhsT = xT8_sb[:, k4 * 256 : (k4 + 1) * 256].rearrange(
                            "p (two b) -> p two b", two=2
                        )
                        for h in range(2):
                            nc.tensor.matmul(
                                e_ps[h][:],
                                lhsT,
                                wt[:, 2 * k4 : 2 * k4 + 2, h * 512 : (h + 1) * 512],
                                start=(k4 == 0),
                                stop=(not with_bias and k4 == KD2 - 1),
                                perf_mode=DR,
                            )
                    e_sb = epool.tile([128, D], XDT, name="e_sb", tag="e_sb")
                    for h in range(2):
                        if with_bias:
                            nc.tensor.matmul(
                                e_ps[h][:],
                                ones_sb[:],
                                brep_sb[:, n * D + h * 512 : n * D + (h + 1) * 512],
                                start=False,
                                stop=True,
                            )
                        nc.vector.tensor_scalar_mul(
                            e_sb[:, h * 512 : (h + 1) * 512],
                            e_ps[h][:],
                            ESCALE / (WSCALE * XSCALE),
                        )
                    # send rows: send[dst, i, :, :] = embed rows of batch
                    # chunk dst (the [128, D] tile viewed as [8, 16, D]).
                    if n < NA:
                        nc.gpsimd.dma_start(send_a[:, n, :, :], e_sb[:])
                    else:
                        nc.gpsimd.dma_start(send_b[:, n - NA, :, :], e_sb[:])
                    if n == NA - 1:
                        nc.gpsimd.collective_compute(
                            "AllToAll",
                            mybir.AluOpType.bypass,
                            replica_groups=[list(range(NCORES))],
                            ins=[send_a[:].opt()],
                            outs=[recv_a[:].opt()],
                        )

            nc.gpsimd.collective_compute(
                "AllToAll",
                mybir.AluOpType.bypass,
                replica_groups=[list(range(NCORES))],
                ins=[send_b[:].opt()],
                outs=[recv_b[:].opt()],
            )

            # recv_a row 5*src+i holds condition 5*src+i (0..40);
            # recv_b row 4*src+j holds condition 40+4*src+j (40..72).
            recva_r = recv_a[:].rearrange("a n b d -> (a n) (b d)")
            recvb_r = recv_b[:].rearrange("a n b d -> (a n) (b d)")

            # ---- off-critical-path work in the a2a-A shadow -------------
            with tc.tile_pool(name="attp", bufs=1, space="PSUM") as attp:
                ceT_ps = attp.tile([CE, N], F32, name="ceT_ps")
                nc.tensor.transpose(ceT_ps[:], ce_sb[:], id_sb[:N, :N])
                nc.vector.tensor_copy(ceT_sb[:], ceT_ps[:])

                h_ps = attp.tile([P, N], F32, name="h_ps")
                nc.tensor.matmul(h_ps[:], ceT_sb[:], w1_sb[:], start=True, stop=False)
                nc.tensor.matmul(
                    h_ps[:], onesA_sb[:, :P], b1_sb[:], start=False, stop=True
                )
                nc.scalar.activation(
                    h_sb[:], h_ps[:], mybir.ActivationFunctionType.Relu
                )

                hT_ps = attp.tile([N, P], F32, name="hT_ps")
                nc.tensor.transpose(hT_ps[:], h_sb[:], id_sb[:P, :P])
                nc.vector.tensor_copy(hT_sb[:], hT_ps[:])

                a_ps = attp.tile([P, N], F32, name="a_ps")
                nc.tensor.matmul(a_ps[:], hT_sb[:], w2_sb[:], start=True, stop=False)
                nc.tensor.matmul(
                    a_ps[:], onesA_sb[:, :P], b2_sb[:], start=False, stop=True
                )
                nc.vector.tensor_copy(att_sb[:], a_ps[:])

                # row softmax
                nc.vector.tensor_reduce(
                    rmax[:], att_sb[:], axis=mybir.AxisListType.X,
                    op=mybir.AluOpType.max,
                )
                nc.vector.tensor_scalar_mul(rmax[:], rmax[:], -1.0)
                nc.scalar.activation(
                    att_sb[:],
                    att_sb[:],
                    mybir.ActivationFunctionType.Exp,
                    bias=rmax[:],
                    accum_out=rsum[:],
                )
                nc.vector.reciprocal(rsum[:], rsum[:])
                nc.vector.tensor_scalar_mul(att_sb[:], att_sb[:], rsum[:])

                # attT40/attT32: bf16 transposes of att columns 0:40 and
                # 40:66 (rows 26..32 of attT32 stay zero for the pad
                # conditions), scaled by 1/ESCALE to undo the exchange
                # scale.
                attTa_ps = attp.tile([CONDA, P], F32, name="attTa_ps")
                nc.tensor.transpose(attTa_ps[:], att_sb[:, :CONDA], id_sb[:P, :P])
                nc.vector.tensor_scalar_mul(attT40[:], attTa_ps[:], 1.0 / ESCALE)
                attTb_ps = attp.tile([N - CONDA, P], F32, name="attTb_ps")
                nc.tensor.transpose(
                    attTb_ps[:], att_sb[:, CONDA:N], id_sb[:P, :P]
                )
                nc.vector.tensor_scalar_mul(
                    attT32[: N - CONDA, :], attTb_ps[:], 1.0 / ESCALE
                )

                # xsrep: this core's 16 x-rows replicated to all 128
                # partitions, via one selection matmul (all-bf16; the
                # selection matrix comes pre-replicated from the host).
                for h in range(2):
                    xs_ps = attp.tile([128, 512], F32, name="xs_ps", tag="xs_ps")
                    nc.tensor.matmul(
                        xs_ps[:],
                        bselrep[:],
                        xbf_sb[:, h * 512 : (h + 1) * 512],
                        start=True,
                        stop=True,
                    )
                    nc.vector.tensor_copy(
                        xsrep_sb[:, h * 512 : (h + 1) * 512], xs_ps[:]
                    )

            # feature_x rows stream out on the gpsimd ring during the a2a
            # window: 9 DMAs of [gc*16, 1024] covering 8 (then 2) slots.
            for m in range(9):
                gc = 8 if m < 8 else 2
                out_ap = out_shard[:, P + 8 * m : P + 8 * m + gc, :].transpose(
                    [1, 0, 2]
                )
                nc.gpsimd.dma_start(out_ap, xsrep_sb[: gc * BL, :])

            # ---- reduce: cond_feat[b,p,:] = sum_n att[p,n] r[n,(b,:)] ---
            # pass 1 contracts conditions 0..40 (overlapping a2a-B), into
            # f32 SBUF accumulators; pass 2 contracts 40..72 and fuses the
            # add on the DVE, emitting bf16 2-batch-row chunks.
            with (
                tc.tile_pool(name="rqpool", bufs=4) as rqpool,
                tc.tile_pool(name="rbpool", bufs=4) as rbpool,
                tc.tile_pool(name="accp", bufs=8) as accp,
                tc.tile_pool(name="rpsum", bufs=4, space="PSUM") as rpsum,
                tc.tile_pool(name="spool", bufs=2) as spool,
            ):
                rqa = []
                for jq in range(4):
                    rq = rqpool.tile([CONDA, 4 * D], XDT, name="rqa", tag="rqa")
                    eng = nc.sync if jq % 2 == 0 else nc.scalar
                    eng.dma_start(rq[:], recva_r[:, jq * 4 * D : (jq + 1) * 4 * D])
                    rqa.append(rq)
                accs = []
                for jq in range(4):
                    for jp in range(2):
                        acc = accp.tile([P, 2 * D], F32, name="acc", tag="acc")
                        for jh in range(4):
                            o_ps = rpsum.tile(
                                [P, 512], F32, name="o_ps", tag="o_ps"
                            )
                            nc.tensor.matmul(
                                o_ps[:],
                                attT40[:],
                                rqa[jq][
                                    :, (jp * 4 + jh) * 512 : (jp * 4 + jh + 1) * 512
                                ],
                                start=True,
                                stop=True,
                            )
                            if jh % 2 == 0:
                                nc.vector.tensor_copy(
                                    acc[:, jh * 512 : (jh + 1) * 512], o_ps[:]
                                )
                            else:
                                nc.scalar.activation(
                                    acc[:, jh * 512 : (jh + 1) * 512],
                                    o_ps[:],
                                    mybir.ActivationFunctionType.Copy,
                                )
                        accs.append(acc)
                rqb = []
                for jq in range(4):
                    rq = rbpool.tile([CONDB, 4 * D], XDT, name="rqb", tag="rqb")
                    eng = nc.sync if jq % 2 == 0 else nc.scalar
                    eng.dma_start(rq[:], recvb_r[:, jq * 4 * D : (jq + 1) * 4 * D])
                    rqb.append(rq)
                for jq in range(4):
                    for jp in range(2):
                        jb2 = jq * 2 + jp
                        acc = accs[jb2]
                        res = spool.tile([P, 2 * D], BF16, name="res", tag="res")
                        for jh in range(4):
                            o_ps = rpsum.tile(
                                [P, 512], F32, name="o_ps", tag="o_ps"
                            )
                            nc.tensor.matmul(
                                o_ps[:],
                                attT32[:],
                                rqb[jq][
                                    :, (jp * 4 + jh) * 512 : (jp * 4 + jh + 1) * 512
                                ],
                                start=True,
                                stop=True,
                            )
                            nc.vector.tensor_add(
                                res[:, jh * 512 : (jh + 1) * 512],
                                acc[:, jh * 512 : (jh + 1) * 512],
                                o_ps[:],
                            )
                        eng = nc.sync if jb2 % 2 == 0 else nc.scalar
                        eng.dma_start(
                            out_shard[jb2 * 2 : (jb2 + 1) * 2, :P, :].transpose(
                                [1, 0, 2]
                            ),
                            res[:].rearrange("p (b d) -> p b d", b=2),
                        )

    _split_multiwait_drains(nc)
    return nc


_NC_CACHE = {}
_LAST_IN_MAPS = None
_WITH_BIAS = False


def _get_nc():
    if _WITH_BIAS not in _NC_CACHE:
        _NC_CACHE[_WITH_BIAS] = _build(_WITH_BIAS)
    return _NC_CACHE[_WITH_BIAS]


def _core_conds(i):
    """Global condition ids owned by core i: A-group then B-group."""
    return list(range(NA * i, NA * i + NA)) + list(
        range(CONDA + NB * i, CONDA + NB * i + NB)
    )


def kernel(image, W_emb, b_emb, W_rep, b_rep, mask_table, W1, b1, W2, b2, cat_enc):
    import ml_dtypes

    image = np.asarray(image, np.float32)
    W_emb = np.asarray(W_emb, np.float32)
    b_emb = np.asarray(b_emb, np.float32).reshape(1, D)
    W_rep = np.asarray(W_rep, np.float32)
    b_rep = np.asarray(b_rep, np.float32)
    mask_table = np.asarray(mask_table, np.float32)
    W1 = np.asarray(W1, np.float32)
    b1 = np.asarray(b1, np.float32).reshape(1, N)
    W2 = np.asarray(W2, np.float32)
    b2 = np.asarray(b2, np.float32).reshape(1, N)
    cat_enc = np.asarray(cat_enc, np.float32)

    # Fold the mask into the per-condition weights/biases
    # (mask*(x@W+b) == x@(W*mask_col) + b*mask), scale by WSCALE for the
    # fp8-e4m3 range (undone on device).  Pad 66 -> 72.
    wrep_pad = np.zeros((NPAD, D, D), np.float32)
    wrep_pad[:N] = W_rep * mask_table[:, None, :] * WSCALE
    brep_pad = np.zeros((NPAD, D), np.float32)
    brep_pad[:N] = b_rep * mask_table * WSCALE * XSCALE
    # pack to the SBUF tile layout: [n][p, k*D+e] = w[n, k*128+p, e]
    wrep_f8 = np.ascontiguousarray(
        wrep_pad.reshape(NPAD, KD, 128, D).transpose(0, 2, 1, 3)
    ).reshape(NPAD, 128, KD * D).astype(ml_dtypes.float8_e4m3)
    brep_bf = brep_pad.astype(ml_dtypes.bfloat16)
    # w_emb packed: [p, k*D+e] = W_emb[k*128+p, e]
    wemb_bf = np.ascontiguousarray(
        W_emb.reshape(KF, 128, D).transpose(1, 0, 2)
    ).reshape(128, KF * D).astype(ml_dtypes.bfloat16)
    # img_t packed: [p, k*128+b] = image[b, k*128+p]
    imgt_bf = np.ascontiguousarray(
        image.T.reshape(KF, 128, B).transpose(1, 0, 2)
    ).reshape(128, KF * B).astype(ml_dtypes.bfloat16)
    bemb_bf = b_emb.astype(ml_dtypes.bfloat16)

    global _WITH_BIAS
    _WITH_BIAS = bool(np.any(b_emb) or np.any(b_rep))
    nc = _get_nc()
    in_maps = []
    for i in range(NCORES):
        conds = _core_conds(i)
        bselrep = np.zeros((B, 128), np.float32)
        for p in range(128):
            bselrep[i * BL + (p % BL), p] = 1.0
        m = {
            "img_t": imgt_bf,
            "w_emb": wemb_bf,
            "w_rep_l": np.ascontiguousarray(wrep_f8[conds]),
            "w1": W1,
            "b1": b1,
            "w2": W2,
            "b2": b2,
            "cat_enc": cat_enc,
            "b_selrep": bselrep.astype(ml_dtypes.bfloat16),
        }
        if _WITH_BIAS:
            m["b_emb"] = bemb_bf
            m["b_rep_l"] = np.ascontiguousarray(brep_bf[conds]).reshape(1, NL * D)
        in_maps.append(m)

    global _LAST_IN_MAPS
    _LAST_IN_MAPS = in_maps
    res = run_bass_kernel_spmd(nc, in_maps, list(range(NCORES)))

    return np.ascontiguousarray(
        np.concatenate(
            [res.results[i]["out_shard"] for i in range(NCORES)], axis=0
        ).astype(np.float32)
    )


# revision 18
# speedup vs baseline: 2.2040x; 1.0968x over previous
"""Trainium2 Bass kernel for ConditionalSimNet2 (moe_routing).

Computation (B=128, FEAT_IN=2048, D=1024, N=P=66 conditions):
    x          = image @ W_emb + b_emb                    [B, D]
    masked_rep = einsum('bd,nde->bne', x, W_rep) + b_rep  [B, N, D]
    embed      = mask_table * masked_rep                  [B, N, D]
    att        = softmax(relu(cat_enc@W1+b1)@W2 + b2)     [P, N]
    cond_feat  = einsum('pn,bnd->bpd', att, embed)        [B, P, D]
    out        = concat([cond_feat, broadcast(x)], 1)     [B, P+N, D]

Sharding: expert-parallel over the 66 conditions on 8 cores (9 each,
zero-padded to 72).  Every core computes x and att redundantly
(cheap), runs its 9 grouped GEMMs against its W_rep shard, exchanges
embed slices over two pipelined fp8 AllToAlls so each core holds all
conditions for its 16-row batch shard, reduces in two K-passes, and
writes its [16, 132, D] output shard (bf16; host concatenates and
upcasts).

The kernel is PE-bound and the board power throttle pins the PE at
K=4/8 (1.2 GHz) for most of the span, so the design cuts PE cycles
and hides the exchange:
  - The grouped GEMM runs fp8e4 x fp8e4 with perf_mode=DoubleRow:
    each matmul contracts K=256 in 512 streaming cycles - half the
    cycles of the bf16/e3m4 path.  Validated: rel err ~4.7e-3 vs the
    2e-2 gate.
  - Conditions are assigned so the exchange splits into two
    collectives over contiguous condition ranges: core i owns
    A: [5i, 5i+5) and B: [40+4i, 40+4i+4).  AllToAll-A fires after
    the 5 A-conditions (overlapping the B-condition GEMMs);
    AllToAll-B fires at GEMM end and overlaps reduce pass 1 (K=40,
    conditions 0..40).  Pass 2 (K=32, conditions 40..72) accumulates
    into pass 1's SBUF result via DVE adds.  This hides most of the
    ~35 us exposed all-to-all window of the single-collective version
    (cross-core HAM-throttle skew makes the barrier expensive).
  - image is pre-transposed AND pre-packed to the SBUF tile layout on
    the host (bf16), removing all 16 image PE-transposes; W_emb/W_rep
    are host-packed so every load is one fully-contiguous DMA.
  - W_rep streams on four rings (gpsimd/vector lead, sync/scalar
    behind the phase-B feeds) so condition 0 lands before the grouped
    GEMM starts.
  - The output is written bf16 (host upcasts): halves the output
    write.  The xsrep/feature_x replicate matmul runs bf16 (f32
    moving operands cost 4 cycles/row on the PE).
  - mask_table is folded into W_rep/b_rep on the host.

Biases are folded into the GEMMs as K=1 matmuls against a ones row
(DVE cannot broadcast across partitions); they are skipped entirely
when the biases are zero (the graded case).
"""

import sys

import numpy as np

try:
    import concourse.bass as bass
except ImportError:  # pragma: no cover - fallback when PYTHONPATH is not set
    sys.path.insert(0, "/opt/trn_rl_repo")
    import concourse.bass as bass

import concourse.mybir as mybir
import concourse.tile as tile
from concourse.bass_utils import run_bass_kernel_spmd
from concourse.masks import make_identity

F32 = mybir.dt.float32
BF16 = mybir.dt.bfloat16
F8 = mybir.dt.float8e3   # e3m4 (exchange payload)
F8E4 = mybir.dt.float8e4  # e4m3 (DoubleRow operands)
DR = mybir.MatmulPerfMode.DoubleRow

# W_rep*mask ships in fp8-e4m3 scaled by WSCALE (absmax ~0.295 -> ~75,
# inside e4m3's 240).  x is quantized on device to fp8-e4m3 scaled by
# XSCALE (absmax ~4.2 -> ~134).  The exchange holds ESCALE*embed in
# e3m4 (max ~14.5 < 15.5); 1/ESCALE folds into the attention lhsT and
# ESCALE/(WSCALE*XSCALE) into the PSUM->send copies.
WSCALE = 256.0
XSCALE = 32.0
ESCALE = 2.0
XDT = F8  # exchange dtype

B = 128          # batch
FI = 2048        # backbone feature dim
D = 1024         # embed dim
N = 66           # conditions (== pair categories P)
P = 66
CE = 24          # 2 * C_CAT
NCORES = 8
NL = 9           # conditions per core (66 -> 72 padded)
NA = 5           # A-group conditions per core (global 0..40)
NB = 4           # B-group conditions per core (global 40..72)
CONDA = NCORES * NA  # 40
CONDB = NCORES * NB  # 32
NPAD = NCORES * NL
BL = B // NCORES  # batch rows per core

KD = D // 128    # 8 k-tiles over D
KD2 = KD // 2    # 4 DoubleRow k-chunks (256-wide) over D
KF = FI // 128   # 16 k-tiles over FEAT_IN

def _split_multiwait_drains(nc):
    """This walrus build only accepts one sem wait per instruction; hoist
    extras onto NoOp carriers inserted just before the instruction (engines
    execute their stream in order, so wait-then-op is equivalent)."""
    fixno = 0
    for fnc in nc.m.functions:
        for bb in fnc.blocks:
            insts = bb.instructions
            i = 0
            while i < len(insts):
                inst = insts[i]
                si = inst.sync_info
                if si is not None and len(si.on_wait) > 1:
                    waits = list(si.on_wait)
                    si.on_wait = waits[-1:]
                    for w in waits[:-1]:
                        fixno += 1
                        carrier = mybir.InstNoOp(
                            name=f"I-waitfix-{fixno}",
                            engine=inst.engine,
                            ins=[],
                            outs=[],
                            sync_info=mybir.SyncInfo(on_wait=[w], on_update=[]),
                        )
                        insts.insert(i, carrier)
                        i += 1
                i += 1
    return fixno


def _build(with_bias):
    nc = bass.Bass(
        "TRN2", target_bir_lowering=False, debug=False, num_devices=NCORES
    )
    ins = {
        # img_t[p, k*128+b] = image[b, k*128+p]  (SBUF tile layout, bf16)
        "img_t": nc.dram_tensor("img_t", [128, KF * 128], BF16, kind="ExternalInput").ap(),
        # w_emb[p, k*D+e] = W_emb[k*128+p, e]    (SBUF tile layout, bf16)
        "w_emb": nc.dram_tensor("w_emb", [128, KF * D], BF16, kind="ExternalInput").ap(),
        # w_rep_l[n][p, k*D+e] = (W_rep*mask*WSCALE)[cond(n), k*128+p, e]
        "w_rep_l": nc.dram_tensor(
            "w_rep_l", [NL, 128, KD * D], F8E4, kind="ExternalInput"
        ).ap(),
        "w1": nc.dram_tensor("w1", [CE, N], F32, kind="ExternalInput").ap(),
        "b1": nc.dram_tensor("b1", [1, N], F32, kind="ExternalInput").ap(),
        "w2": nc.dram_tensor("w2", [N, N], F32, kind="ExternalInput").ap(),
        "b2": nc.dram_tensor("b2", [1, N], F32, kind="ExternalInput").ap(),
        "cat_enc": nc.dram_tensor("cat_enc", [N, CE], F32, kind="ExternalInput").ap(),
        # host-built row-selection matrix replicated 8x: selects and
        # replicates this core's 16 x-rows to all 128 partitions (built on
        # the host so no DVE copies sit ahead of the critical xT8 scales
        # in the strict-FIFO vector queue)
        "b_selrep": nc.dram_tensor(
            "b_selrep", [B, 128], BF16, kind="ExternalInput"
        ).ap(),
    }
    if with_bias:
        ins["b_emb"] = nc.dram_tensor(
            "b_emb", [1, D], BF16, kind="ExternalInput"
        ).ap()
        ins["b_rep_l"] = nc.dram_tensor(
            "b_rep_l", [1, NL * D], BF16, kind="ExternalInput"
        ).ap()
    send_a = nc.dram_tensor("a2a_send_a", [NCORES, NA, BL, D], XDT)
    recv_a = nc.dram_tensor("a2a_recv_a", [NCORES, NA, BL, D], XDT)
    send_b = nc.dram_tensor("a2a_send_b", [NCORES, NB, BL, D], XDT)
    recv_b = nc.dram_tensor("a2a_recv_b", [NCORES, NB, BL, D], XDT)
    out_shard = nc.dram_tensor(
        "out_shard", [BL, P + N, D], BF16, kind="ExternalOutput"
    ).ap()

    with tile.TileContext(nc) as tc, tc.tile_pool(name="const", bufs=1) as cpool:
        # ---- persistent tiles --------------------------------------------
        id_sb = cpool.tile([128, 128], F32, name="id_sb")
        id_bf = cpool.tile([128, 128], BF16, name="id_bf")
        if with_bias:
            bemb_sb = cpool.tile([1, D], BF16, name="bemb_sb")
            brep_sb = cpool.tile([1, NL * D], BF16, name="brep_sb")
        # all 9 conditions' weights stay SBUF-resident (72 KiB/partition
        # in fp8): nine independent DMAs, no ring-reuse deps.
        w_all = cpool.tile([128, NL * KD * D], F8E4, name="w_all")
        ce_sb = cpool.tile([N, CE], F32, name="ce_sb")
        w1_sb = cpool.tile([CE, N], F32, name="w1_sb")
        b1_sb = cpool.tile([1, N], F32, name="b1_sb")
        w2_sb = cpool.tile([N, N], F32, name="w2_sb")
        b2_sb = cpool.tile([1, N], F32, name="b2_sb")
        bselrep = cpool.tile([B, 128], BF16, name="bselrep")
        onesA_sb = cpool.tile([1, 128], F32, name="onesA_sb")
        ones_sb = cpool.tile([1, 128], BF16, name="ones_sb")
        xbf_sb = cpool.tile([128, D], BF16, name="xbf_sb")
        xT8_sb = cpool.tile([128, D], F8E4, name="xT8_sb")
        attT72 = cpool.tile([NPAD, P], BF16, name="attT72")
        ceT_sb = cpool.tile([CE, N], F32, name="ceT_sb")
        h_sb = cpool.tile([P, N], F32, name="h_sb")
        hT_sb = cpool.tile([N, P], F32, name="hT_sb")
        att_sb = cpool.tile([P, N], F32, name="att_sb")
        rmax = cpool.tile([P, 1], F32, name="rmax")
        rsum = cpool.tile([P, 1], F32, name="rsum")

        with (
            tc.tile_pool(name="bpool", bufs=1) as bpool,
            tc.tile_pool(name="bpsum", bufs=2, space="PSUM") as bpsum,
            tc.tile_pool(name="tpsum", bufs=2, space="PSUM") as tpsum,
        ):
            imgT_sb = bpool.tile([128, KF * 128], BF16, name="imgT_sb")
            we_sb = bpool.tile([128, KF * D], BF16, name="we_sb")

            # ---- DMA issue phase: ring order == need order --------------
            # Three rings (sync/scalar/gpsimd) each carry an interleaved
            # slice of the phase-B feed (k-chunk round-robin so the x
            # matmuls stream without starving), then the W_rep conditions
            # in need order: gpsimd (otherwise idle) leads with n0..n2.
            nc.sync.dma_start(imgT_sb[:, : 4 * 128], ins["img_t"][:, : 4 * 128])
            nc.sync.dma_start(we_sb[:, : 2 * D], ins["w_emb"][:, : 2 * D])
            nc.scalar.dma_start(we_sb[:, 2 * D : 4 * D], ins["w_emb"][:, 2 * D : 4 * D])
            nc.gpsimd.dma_start(we_sb[:, 4 * D : 6 * D], ins["w_emb"][:, 4 * D : 6 * D])
            nc.scalar.dma_start(imgT_sb[:, 4 * 128 :], ins["img_t"][:, 4 * 128 :])
            nc.sync.dma_start(we_sb[:, 6 * D : 8 * D], ins["w_emb"][:, 6 * D : 8 * D])
            nc.scalar.dma_start(we_sb[:, 8 * D : 10 * D], ins["w_emb"][:, 8 * D : 10 * D])
            nc.gpsimd.dma_start(we_sb[:, 10 * D : 12 * D], ins["w_emb"][:, 10 * D : 12 * D])
            nc.sync.dma_start(we_sb[:, 12 * D : 14 * D], ins["w_emb"][:, 12 * D : 14 * D])
            nc.scalar.dma_start(we_sb[:, 14 * D : 16 * D], ins["w_emb"][:, 14 * D : 16 * D])
            for n in range(3):
                nc.gpsimd.dma_start(
                    w_all[:, n * KD * D : (n + 1) * KD * D], ins["w_rep_l"][n]
                )
            for n in range(3, NL):
                eng = nc.scalar if n % 2 == 0 else nc.sync
                eng.dma_start(
                    w_all[:, n * KD * D : (n + 1) * KD * D], ins["w_rep_l"][n]
                )
            nc.sync.dma_start(ce_sb[:], ins["cat_enc"][:])
            nc.sync.dma_start(w1_sb[:], ins["w1"][:])
            nc.sync.dma_start(b1_sb[:], ins["b1"][:])
            nc.scalar.dma_start(w2_sb[:], ins["w2"][:])
            nc.scalar.dma_start(b2_sb[:], ins["b2"][:])
            nc.scalar.dma_start(bselrep[:], ins["b_selrep"][:])
            if with_bias:
                nc.scalar.dma_start(bemb_sb[:], ins["b_emb"][:])
                nc.scalar.dma_start(brep_sb[:], ins["b_rep_l"][:])

            # constants.  onesA lands on the (otherwise idle) DVE so the
            # PE warmup below can start the moment the preamble ends; the
            # rest go on gpsimd after its DMA issues.
            nc.vector.memset(onesA_sb[:], 1.0)
            make_identity(nc, id_sb[:])
            make_identity(nc, id_bf[:])
            nc.gpsimd.memset(ones_sb[:], 1.0)
            nc.gpsimd.memset(attT72[:], 0.0)

            # PE warmup: ~10 junk matmuls on the ones row, issued while the
            # first input DMAs are still in flight.  The HAM clock gate
            # needs ~3.4us of sustained PE activity to lift the PE from
            # 1.2 to 2.4 GHz; without this, all of phase B (and the HAM
            # window into phase C) runs at half clock.
            with tc.tile_pool(name="wpsum", bufs=2, space="PSUM") as wpsum:
                for w in range(10):
                    wps = wpsum.tile([128, 128], F32, name="wps", tag="wps")
                    nc.tensor.matmul(
                        wps[:], onesA_sb[:], onesA_sb[:], start=True, stop=True
                    )

            # ---- phase B: x = image @ W_emb (+ b_emb), xT8 --------------
            x_ps = [bpsum.tile([128, 512], F32, name=f"x_ps{h}") for h in range(2)]
            for k in range(KF):
                for h in range(2):
                    nc.tensor.matmul(
                        x_ps[h][:],
                        imgT_sb[:, k * 128 : (k + 1) * 128],
                        we_sb[:, k * D + h * 512 : k * D + (h + 1) * 512],
                        start=(k == 0),
                        stop=(not with_bias and k == KF - 1),
                    )
            for h in range(2):
                if with_bias:
                    nc.tensor.matmul(
                        x_ps[h][:],
                        ones_sb[:],
                        bemb_sb[:, h * 512 : (h + 1) * 512],
                        start=False,
                        stop=True,
                    )
                # both halves on the DVE: the ACT engine's strict FIFO is
                # full of scalar-ring dma_start issue ops (which block on
                # semaphore-pool reuse) and would stall this copy - and
                # with it the transposes and all of phase C - by ~12 us.
                nc.vector.tensor_copy(
                    xbf_sb[:, h * 512 : (h + 1) * 512], x_ps[h][:]
                )
            for m in range(KD):
                tpb = tpsum.tile([128, 128], BF16, name="tpb", tag="tpb")
                nc.tensor.transpose(
                    tpb[:], xbf_sb[:, m * 128 : (m + 1) * 128], id_bf[:]
                )
                nc.vector.tensor_scalar_mul(
                    xT8_sb[:, m * 128 : (m + 1) * 128], tpb[:], XSCALE
                )

        with tc.tile_pool(name="rpool", bufs=1) as rpool:
            xsrep_sb = rpool.tile([128, D], BF16, name="xsrep_sb")

            # ---- phase C: grouped GEMM over the 9 local conditions ------
            # DoubleRow fp8e4: each matmul contracts a 256-wide k-chunk
            # (two stacked 128-tiles along the free axis of both operands)
            # in 512 streaming cycles.  AllToAll-A fires after condition 4.
            with (
                tc.tile_pool(name="epool", bufs=3) as epool,
                tc.tile_pool(name="cpsum", bufs=4, space="PSUM") as cpsum,
            ):
                for n in range(NL):
                    wt = w_all[:, n * KD * D : (n + 1) * KD * D].rearrange(
                        "p (k d) -> p k d", k=KD
                    )
                    e_ps = [
                        cpsum.tile([128, 512], F32, name="e_ps", tag=f"e_ps{h}")
                        for h in range(2)
                    ]
                    for k4 in range(KD2):
                        lhsT = xT8_sb[:, k4 * 256 : (k4 + 1) * 256].rearrange(
                            "p (two b) -> p two b", two=2
                        )
                        for h in range(2):
                            nc.tensor.matmul(
                                e_ps[h][:],
                                lhsT,
                                wt[:, 2 * k4 : 2 * k4 + 2, h * 512 : (h + 1) * 512],
                                start=(k4 == 0),
                                stop=(not with_bias and k4 == KD2 - 1),
                                perf_mode=DR,
                            )
                    e_sb = epool.tile([128, D], XDT, name="e_sb", tag="e_sb")
                    for h in range(2):
                        if with_bias:
                            nc.tensor.matmul(
                                e_ps[h][:],
                                ones_sb[:],
                                brep_sb[:, n * D + h * 512 : n * D + (h + 1) * 512],
                                start=False,
                                stop=True,
                            )
                        nc.vector.tensor_scalar_mul(
                            e_sb[:, h * 512 : (h + 1) * 512],
                            e_ps[h][:],
                            ESCALE / (WSCALE * XSCALE),
                        )
                    # send rows: send[dst, i, :, :] = embed rows of batch
                    # chunk dst (the [128, D] tile viewed as [8, 16, D]).
                    if n < NA:
                        nc.gpsimd.dma_start(send_a[:, n, :, :], e_sb[:])
                    else:
                        nc.gpsimd.dma_start(send_b[:, n - NA, :, :], e_sb[:])
                    if n == NA - 1:
                        nc.gpsimd.collective_compute(
                            "AllToAll",
                            mybir.AluOpType.bypass,
                            replica_groups=[list(range(NCORES))],
                            ins=[send_a[:].opt()],
                            outs=[recv_a[:].opt()],
                        )

            nc.gpsimd.collective_compute(
                "AllToAll",
                mybir.AluOpType.bypass,
                replica_groups=[list(range(NCORES))],
                ins=[send_b[:].opt()],
                outs=[recv_b[:].opt()],
            )

            # recv_a row 5*src+i holds condition 5*src+i (0..40);
            # recv_b row 4*src+j holds condition 40+4*src+j (40..72).
            recva_r = recv_a[:].rearrange("a n b d -> (a n) (b d)")
            recvb_r = recv_b[:].rearrange("a n b d -> (a n) (b d)")

            # ---- off-critical-path work in the a2a-A shadow -------------
            with tc.tile_pool(name="attp", bufs=1, space="PSUM") as attp:
                ceT_ps = attp.tile([CE, N], F32, name="ceT_ps")
                nc.tensor.transpose(ceT_ps[:], ce_sb[:], id_sb[:N, :N])
                nc.vector.tensor_copy(ceT_sb[:], ceT_ps[:])

                h_ps = attp.tile([P, N], F32, name="h_ps")
                nc.tensor.matmul(h_ps[:], ceT_sb[:], w1_sb[:], start=True, stop=False)
                nc.tensor.matmul(
                    h_ps[:], onesA_sb[:, :P], b1_sb[:], start=False, stop=True
                )
                nc.scalar.activation(
                    h_sb[:], h_ps[:], mybir.ActivationFunctionType.Relu
                )

                hT_ps = attp.tile([N, P], F32, name="hT_ps")
                nc.tensor.transpose(hT_ps[:], h_sb[:], id_sb[:P, :P])
                nc.vector.tensor_copy(hT_sb[:], hT_ps[:])

                a_ps = attp.tile([P, N], F32, name="a_ps")
                nc.tensor.matmul(a_ps[:], hT_sb[:], w2_sb[:], start=True, stop=False)
                nc.tensor.matmul(
                    a_ps[:], onesA_sb[:, :P], b2_sb[:], start=False, stop=True
                )
                nc.vector.tensor_copy(att_sb[:], a_ps[:])

                # row softmax
                nc.vector.tensor_reduce(
                    rmax[:], att_sb[:], axis=mybir.AxisListType.X,
                    op=mybir.AluOpType.max,
                )
                nc.vector.tensor_scalar_mul(rmax[:], rmax[:], -1.0)
                nc.scalar.activation(
                    att_sb[:],
                    att_sb[:],
                    mybir.ActivationFunctionType.Exp,
                    bias=rmax[:],
                    accum_out=rsum[:],
                )
                nc.vector.reciprocal(rsum[:], rsum[:])
                nc.vector.tensor_scalar_mul(att_sb[:], att_sb[:], rsum[:])

                # attT72: zero-padded bf16 transpose of att, scaled by
                # 1/ESCALE to undo the exchange scale.  With the A/B
                # condition assignment, recv_a rows are conditions 0..40
                # and recv_b rows are 40..72, so att columns transpose
                # straight into condition-order rows.
                attT_ps = attp.tile([N, P], F32, name="attT_ps")
                nc.tensor.transpose(attT_ps[:], att_sb[:], id_sb[:P, :P])
                nc.vector.tensor_scalar_mul(attT72[:N, :], attT_ps[:], 1.0 / ESCALE)

                # xsrep: this core's 16 x-rows replicated to all 128
                # partitions, via one selection matmul (all-bf16; the
                # selection matrix comes pre-replicated from the host).
                for h in range(2):
                    xs_ps = attp.tile([128, 512], F32, name="xs_ps", tag="xs_ps")
                    nc.tensor.matmul(
                        xs_ps[:],
                        bselrep[:],
                        xbf_sb[:, h * 512 : (h + 1) * 512],
                        start=True,
                        stop=True,
                    )
                    nc.vector.tensor_copy(
                        xsrep_sb[:, h * 512 : (h + 1) * 512], xs_ps[:]
                    )

            # feature_x rows stream out on the gpsimd ring during the a2a
            # window: 9 DMAs of [gc*16, 1024] covering 8 (then 2) slots.
            for m in range(9):
                gc = 8 if m < 8 else 2
                out_ap = out_shard[:, P + 8 * m : P + 8 * m + gc, :].transpose(
                    [1, 0, 2]
                )
                nc.gpsimd.dma_start(out_ap, xsrep_sb[: gc * BL, :])

            # PE bridge: junk matmuls that keep the HAM clock gate at
            # 8/8 through the exchange window so the reduce matmuls run
            # at 2.4 GHz instead of re-throttled 1.2.
            with tc.tile_pool(name="jpsum", bufs=2, space="PSUM") as jpsum:
                for w in range(36):
                    jps = jpsum.tile([128, 128], F32, name="jps", tag="jps")
                    nc.tensor.matmul(
                        jps[:], onesA_sb[:], onesA_sb[:], start=True, stop=True
                    )

            # ---- reduce: cond_feat[b,p,:] = sum_n att[p,n] r[n,(b,:)] ---
            # one K=72 pass per column block (matmul cost is moving
            # columns, so a single pass over the combined A+B rows costs
            # half of two per-group passes).  Each quarter tile is filled
            # by two DMAs: rows 0:40 from recv_a (sync ring, gated on
            # a2a-A) and rows 40:72 from recv_b (scalar ring, gated on
            # a2a-B); the matmuls wait on both.
            with (
                tc.tile_pool(name="rqpool", bufs=4) as rqpool,
                tc.tile_pool(name="rpsum", bufs=4, space="PSUM") as rpsum,
                tc.tile_pool(name="spool", bufs=2) as spool,
            ):
                rqs = []
                for jq in range(4):
                    rq = rqpool.tile([NPAD, 4 * D], XDT, name="rq", tag="rq")
                    nc.sync.dma_start(
                        rq[:CONDA, :], recva_r[:, jq * 4 * D : (jq + 1) * 4 * D]
                    )
                    nc.scalar.dma_start(
                        rq[CONDA:, :], recvb_r[:, jq * 4 * D : (jq + 1) * 4 * D]
                    )
                    rqs.append(rq)
                for jq in range(4):
                    for jp in range(2):
                        jb2 = jq * 2 + jp
                        res = spool.tile([P, 2 * D], BF16, name="res", tag="res")
                        for jh in range(4):
                            o_ps = rpsum.tile(
                                [P, 512], F32, name="o_ps", tag="o_ps"
                            )
                            nc.tensor.matmul(
                                o_ps[:],
                                attT72[:],
                                rqs[jq][
                                    :, (jp * 4 + jh) * 512 : (jp * 4 + jh + 1) * 512
                                ],
                                start=True,
                                stop=True,
                            )
                            if jh % 2 == 0:
                                nc.vector.tensor_copy(
                                    res[:, jh * 512 : (jh + 1) * 512], o_ps[:]
                                )
                            else:
                                nc.scalar.activation(
                                    res[:, jh * 512 : (jh + 1) * 512],
                                    o_ps[:],
                                    mybir.ActivationFunctionType.Copy,
                                )
                        eng = nc.sync if jb2 % 2 == 0 else nc.scalar
                        eng.dma_start(
                            out_shard[jb2 * 2 : (jb2 + 1) * 2, :P, :].transpose(
                                [1, 0, 2]
                            ),
                            res[:].rearrange("p (b d) -> p b d", b=2),
                        )

    _split_multiwait_drains(nc)
    return nc


_NC_CACHE = {}
_LAST_IN_MAPS = None
_WITH_BIAS = False


def _get_nc():
    if _WITH_BIAS not in _NC_CACHE:
        _NC_CACHE[_WITH_BIAS] = _build(_WITH_BIAS)
    return _NC_CACHE[_WITH_BIAS]


def _core_conds(i):
    """Global condition ids owned by core i: A-group then B-group."""
    return list(range(NA * i, NA * i + NA)) + list(
        range(CONDA + NB * i, CONDA + NB * i + NB)
    )


def kernel(image, W_emb, b_emb, W_rep, b_rep, mask_table, W1, b1, W2, b2, cat_enc):
    import ml_dtypes

    image = np.asarray(image, np.float32)
    W_emb = np.asarray(W_emb, np.float32)
    b_emb = np.asarray(b_emb, np.float32).reshape(1, D)
    W_rep = np.asarray(W_rep, np.float32)
    b_rep = np.asarray(b_rep, np.float32)
    mask_table = np.asarray(mask_table, np.float32)
    W1 = np.asarray(W1, np.float32)
    b1 = np.asarray(b1, np.float32).reshape(1, N)
    W2 = np.asarray(W2, np.float32)
    b2 = np.asarray(b2, np.float32).reshape(1, N)
    cat_enc = np.asarray(cat_enc, np.float32)

    # Fold the mask into the per-condition weights/biases
    # (mask*(x@W+b) == x@(W*mask_col) + b*mask), scale by WSCALE for the
    # fp8-e4m3 range (undone on device).  Pad 66 -> 72.
    wrep_pad = np.zeros((NPAD, D, D), np.float32)
    wrep_pad[:N] = W_rep * mask_table[:, None, :] * WSCALE
    brep_pad = np.zeros((NPAD, D), np.float32)
    brep_pad[:N] = b_rep * mask_table * WSCALE * XSCALE
    # pack to the SBUF tile layout: [n][p, k*D+e] = w[n, k*128+p, e]
    wrep_f8 = np.ascontiguousarray(
        wrep_pad.reshape(NPAD, KD, 128, D).transpose(0, 2, 1, 3)
    ).reshape(NPAD, 128, KD * D).astype(ml_dtypes.float8_e4m3)
    brep_bf = brep_pad.astype(ml_dtypes.bfloat16)
    # w_emb packed: [p, k*D+e] = W_emb[k*128+p, e]
    wemb_bf = np.ascontiguousarray(
        W_emb.reshape(KF, 128, D).transpose(1, 0, 2)
    ).reshape(128, KF * D).astype(ml_dtypes.bfloat16)
    # img_t packed: [p, k*128+b] = image[b, k*128+p]
    imgt_bf = np.ascontiguousarray(
        image.T.reshape(KF, 128, B).transpose(1, 0, 2)
    ).reshape(128, KF * B).astype(ml_dtypes.bfloat16)
    bemb_bf = b_emb.astype(ml_dtypes.bfloat16)

    global _WITH_BIAS
    _WITH_BIAS = bool(np.any(b_emb) or np.any(b_rep))
    nc = _get_nc()
    in_maps = []
    for i in range(NCORES):
        conds = _core_conds(i)
        bselrep = np.zeros((B, 128), np.float32)
        for p in range(128):
            bselrep[i * BL + (p % BL), p] = 1.0
        m = {
            "img_t": imgt_bf,
            "w_emb": wemb_bf,
            "w_rep_l": np.ascontiguousarray(wrep_f8[conds]),
            "w1": W1,
            "b1": b1,
            "w2": W2,
            "b2": b2,
            "cat_enc": cat_enc,
            "b_selrep": bselrep.astype(ml_dtypes.bfloat16),
        }
        if _WITH_BIAS:
            m["b_emb"] = bemb_bf
            m["b_rep_l"] = np.ascontiguousarray(brep_bf[conds]).reshape(1, NL * D)
        in_maps.append(m)

    global _LAST_IN_MAPS
    _LAST_IN_MAPS = in_maps
    res = run_bass_kernel_spmd(nc, in_maps, list(range(NCORES)))

    return np.ascontiguousarray(
        np.concatenate(
            [res.results[i]["out_shard"] for i in range(NCORES)], axis=0
        ).astype(np.float32)
    )
